# revision 25
# baseline (speedup 1.0000x reference)
"""Trainium2 Bass kernel for nn_AdaptedEditFlowsTransformer (8 NeuronCores, TP8).

Sharding: tensor-parallel over 8 cores — 2 attention heads/core, d_ff 512/core,
vocab 4000/core. Residual replicated feature-major [H, T]; AllReduce after Wo
and Wd partials (bf16), AllGather for ins/sub head hidden, one tiny AllReduce
for the vocab-softmax denominators. Matmuls bf16 with f32 PSUM accumulation.
"""
import sys
sys.path.insert(0, "/opt/trn_rl_repo")

import numpy as np
from contextlib import ExitStack

import concourse.bass as bass
import concourse.tile as tile
from concourse import bacc, mybir
from concourse.masks import make_identity

B, S, C, L = 2, 512, 512, 1024
H, NH, HD, F, NL, V, TD = 1024, 16, 64, 4096, 4, 32000, 512
NC_ = 8
HPC = NH // NC_
FS = F // NC_          # 512
VS = V // NC_          # 4000
T = B * L              # 2048
TS = B * S             # 1024
HTD = H + TD
HC = H // 128          # 8
F32 = mybir.dt.float32
BF16 = mybir.dt.bfloat16
I32 = mybir.dt.int32
Exp = mybir.ActivationFunctionType.Exp
Sigmoid = mybir.ActivationFunctionType.Sigmoid
Sqrt = mybir.ActivationFunctionType.Sqrt
Copy = mybir.ActivationFunctionType.Copy
Alu = mybir.AluOpType
AX = mybir.AxisListType.X


# ------------------------------------------------------------------ host prep
def _rope_tables():
    inv_freq = 1.0 / (10000.0 ** (np.arange(0, HD, 2, dtype=np.float64) / HD))
    ang = np.arange(L, dtype=np.float64)[:, None] * inv_freq
    cos = np.concatenate([np.cos(ang), np.cos(ang)], axis=-1)
    sin2 = np.concatenate([-np.sin(ang), np.sin(ang)], axis=-1)   # sign-folded
    cos_fm = np.tile(cos.T, (HPC, B)).astype(np.float32)          # [128, T]
    sin_fm = np.tile(sin2.T, (HPC, B)).astype(np.float32)
    return cos_fm, sin_fm


def _time_embed(t):
    half = TD // 2
    freqs = np.exp(-np.log(10000.0) * np.arange(half, dtype=np.float32) / half)
    ang = np.asarray(t, np.float32)[:, None] * freqs[None, :]
    return np.concatenate([np.sin(ang), np.cos(ang)], axis=-1).astype(np.float32)


def host_prep(inputs):
    f32 = np.float32
    import ml_dtypes
    bf = lambda x: np.asarray(x, f32).astype(ml_dtypes.bfloat16)

    tokens = np.asarray(inputs["tokens"])
    context = np.asarray(inputs["context_tokens"])
    pad_tok = int(np.asarray(inputs["pad_token"]))
    combined = np.concatenate([context, tokens], axis=1).astype(np.int32)
    flat = combined.reshape(T)
    tok_pm = np.ascontiguousarray(flat.reshape(16, 128).T)
    tok_free = np.ascontiguousarray(flat.reshape(1, T))

    cos_fm, sin_fm = _rope_tables()
    te = _time_embed(np.asarray(inputs["t"]))
    m_flat = (~np.asarray(inputs["pad_mask"])).astype(f32).reshape(TS)
    m_pm = np.ascontiguousarray(m_flat.reshape(8, 128).T)
    causal_T = (np.arange(L)[None, :] >= np.arange(L)[:, None]).astype(f32)
    rand_T = np.ascontiguousarray(np.asarray(inputs["rand_unif"], f32)[:, 0].transpose(0, 2, 1))

    per_core = []
    for c in range(NC_):
        hs = slice(c * 128, (c + 1) * 128)
        fs = slice(c * FS, (c + 1) * FS)
        vs = slice(c * VS, (c + 1) * VS)
        m = dict(
            tok_pm=tok_pm, tok_free=tok_free,
            embed=np.asarray(inputs["embed"], f32),
            rand_T=rand_T,
            causal_T=bf(causal_T),
            cos_t=bf(cos_fm), sin_t=bf(sin_fm),
            te=te, m_pm=m_pm,
            wq=np.ascontiguousarray(np.asarray(inputs["Wq"], f32)[:, :, hs]),
            wk=np.ascontiguousarray(np.asarray(inputs["Wk"], f32)[:, :, hs]),
            wv=np.ascontiguousarray(np.asarray(inputs["Wv"], f32)[:, :, hs]),
            wo=np.ascontiguousarray(np.asarray(inputs["Wo"], f32)[:, hs, :]),
            wg=np.ascontiguousarray(np.asarray(inputs["Wg"], f32)[:, :, fs]),
            wu=np.ascontiguousarray(np.asarray(inputs["Wu"], f32)[:, :, fs]),
            wd=np.ascontiguousarray(np.asarray(inputs["Wd"], f32)[:, fs, :]),
            ln1=np.asarray(inputs["ln1"], f32), ln2=np.asarray(inputs["ln2"], f32),
            final_ln=np.asarray(inputs["final_ln"], f32),
            insw1=np.ascontiguousarray(np.asarray(inputs["ins_w1"], f32)[:, hs]),
            subw1=np.ascontiguousarray(np.asarray(inputs["sub_w1"], f32)[:, hs]),
            insb1=np.ascontiguousarray(np.asarray(inputs["ins_b1"], f32)[hs]),
            subb1=np.ascontiguousarray(np.asarray(inputs["sub_b1"], f32)[hs]),
            insw2=np.ascontiguousarray(np.asarray(inputs["ins_w2"], f32)[:, vs]),
            subw2=np.ascontiguousarray(np.asarray(inputs["sub_w2"], f32)[:, vs]),
            insb2=np.ascontiguousarray(np.asarray(inputs["ins_b2"], f32)[vs].reshape(1, VS)),
            subb2=np.ascontiguousarray(np.asarray(inputs["sub_b2"], f32)[vs].reshape(1, VS)),
            ratew1=np.asarray(inputs["rate_w1"], f32),
            rateb1=np.asarray(inputs["rate_b1"], f32),
            ratew2=np.asarray(inputs["rate_w2"], f32),
            rateb2=np.asarray(inputs["rate_b2"], f32).reshape(1, 3),
        )
        per_core.append(m)
    return pad_tok, per_core


# -------------------------------------------------------------- device build
def build(pad_tok):
    nc = bacc.Bacc(None, target_bir_lowering=False, num_devices=NC_)
    ctx = ExitStack()

    def ein(name, shape, dtype):
        return nc.dram_tensor(name, shape, dtype, kind="ExternalInput")

    tok_pm_e = ein("tok_pm", [128, 16], I32)
    tok_free_e = ein("tok_free", [1, T], I32)
    embed_e = ein("embed", [V, H], F32)
    rand_e = ein("rand_T", [B, L, L], F32)
    causal_e = ein("causal_T", [L, L], BF16)
    cos_e = ein("cos_t", [128, T], BF16)
    sin_e = ein("sin_t", [128, T], BF16)
    te_e = ein("te", [B, TD], F32)
    m_pm_e = ein("m_pm", [128, 8], F32)
    wq_e = ein("wq", [NL, H, 128], F32)
    wk_e = ein("wk", [NL, H, 128], F32)
    wv_e = ein("wv", [NL, H, 128], F32)
    wo_e = ein("wo", [NL, 128, H], F32)
    wg_e = ein("wg", [NL, H, FS], F32)
    wu_e = ein("wu", [NL, H, FS], F32)
    wd_e = ein("wd", [NL, FS, H], F32)
    ln1_e = ein("ln1", [NL, H], F32)
    ln2_e = ein("ln2", [NL, H], F32)
    fln_e = ein("final_ln", [H], F32)
    insw1_e = ein("insw1", [HTD, 128], F32)
    subw1_e = ein("subw1", [HTD, 128], F32)
    insb1_e = ein("insb1", [128], F32)
    subb1_e = ein("subb1", [128], F32)
    insw2_e = ein("insw2", [H, VS], F32)
    subw2_e = ein("subw2", [H, VS], F32)
    insb2_e = ein("insb2", [1, VS], F32)
    subb2_e = ein("subb2", [1, VS], F32)
    ratew1_e = ein("ratew1", [HTD, TD], F32)
    rateb1_e = ein("rateb1", [TD], F32)
    ratew2_e = ein("ratew2", [TD, 3], F32)
    rateb2_e = ein("rateb2", [1, 3], F32)

    rates_o = nc.dram_tensor("rates_out", [TS, 3], F32, kind="ExternalOutput")
    ins_o = nc.dram_tensor("ins_out", [TS, VS], F32, kind="ExternalOutput")
    sub_o = nc.dram_tensor("sub_out", [TS, VS], F32, kind="ExternalOutput")

    RG = [list(range(NC_))]
    NT = T // 512   # 4 token chunks of 512

    with tile.TileContext(nc) as tc:
        ctx.enter_context(nc.allow_low_precision(reason="bf16 compute; 2e-2 tolerance"))
        consts = ctx.enter_context(tc.tile_pool(name="consts", bufs=1))
        dram = ctx.enter_context(tc.tile_pool(name="dram", bufs=1, space="DRAM"))
        trunk_ctx = ExitStack()
        trunk = trunk_ctx.enter_context(tc.tile_pool(name="trunk", bufs=1))

        ident_b = consts.tile([128, 128], BF16, name="ident_b")
        make_identity(nc, ident_b)
        ident_f = consts.tile([128, 128], F32, name="ident_f")
        make_identity(nc, ident_f)
        ones_b = consts.tile([128, 1], BF16, name="ones_b")
        nc.vector.memset(ones_b, 1.0)
        onesrow_b = consts.tile([1, 128], BF16, name="onesrow_b")
        nc.vector.memset(onesrow_b, 1.0)
        eps_t = consts.tile([128, 1], F32, name="eps_t")
        nc.vector.memset(eps_t, 1e-6)

        cos_s = trunk.tile([128, T], BF16, name="cos_s")
        sin_s = trunk.tile([128, T], BF16, name="sin_s")
        nc.sync.dma_start(out=cos_s, in_=cos_e[:, :])
        nc.sync.dma_start(out=sin_s, in_=sin_e[:, :])
        tok_pm_s = consts.tile([128, 16], I32, name="tok_pm_s")
        nc.sync.dma_start(out=tok_pm_s, in_=tok_pm_e[:, :])
        m_pm_s = consts.tile([128, 8], F32, name="m_pm_s")
        nc.sync.dma_start(out=m_pm_s, in_=m_pm_e[:, :])

        # ------------- mask bias -> DRAM [B, 8, 128, L] bf16 -------------
        bias_d = dram.tile([B, 8, 128, L], BF16, name="bias_d", tag="bias_d")
        with tc.tile_pool(name="maskp", bufs=2) as mp:
            tok_free_s = mp.tile([1, T], I32, name="tok_free_s", tag="tokf", bufs=1)
            nc.sync.dma_start(out=tok_free_s, in_=tok_free_e[:, :])
            padokk = consts.tile([128, 16], F32, name="padokk")
            nc.vector.tensor_scalar(padokk, tok_pm_s, float(pad_tok), None, Alu.not_equal)
            padokq_row = mp.tile([1, T], F32, name="padokq_row", tag="pqr", bufs=1)
            nc.vector.tensor_scalar(padokq_row, tok_free_s, float(pad_tok), None, Alu.not_equal)
            padokq = mp.tile([128, T], F32, name="padokq", tag="pq", bufs=1)
            nc.gpsimd.partition_broadcast(padokq, padokq_row)
            for b in range(B):
                for kc in range(8):
                    rnd = mp.tile([128, L], F32, name="rnd", tag="rnd")
                    nc.sync.dma_start(out=rnd, in_=rand_e[b, kc * 128:(kc + 1) * 128, :])
                    cau = mp.tile([128, L], BF16, name="cau", tag="cau")
                    nc.sync.dma_start(out=cau, in_=causal_e[kc * 128:(kc + 1) * 128, :])
                    t0 = mp.tile([128, L], F32, name="t0", tag="mask0")
                    nc.vector.tensor_scalar(t0, rnd, 0.1, None, Alu.is_lt)
                    t1 = mp.tile([128, L], F32, name="t1", tag="mask1")
                    nc.vector.tensor_tensor(out=t1, in0=t0, in1=cau, op=Alu.max)
                    t2 = mp.tile([128, L], F32, name="t2", tag="mask2")
                    nc.vector.scalar_tensor_tensor(
                        out=t2, in0=t1, scalar=padokk[:, b * 8 + kc:b * 8 + kc + 1],
                        in1=padokq[:, b * L:(b + 1) * L], op0=Alu.mult, op1=Alu.mult)
                    bb = mp.tile([128, L], BF16, name="bb", tag="bb")
                    nc.scalar.activation(bb, t2, Copy, bias=-30.0, scale=30.0)
                    nc.sync.dma_start(out=bias_d[b, kc], in_=bb)

        # ------------- embedding gather -> x_fm (feature-major f32) -------
        x_fm = [trunk.tile([128, T], F32, name=f"x_{hc}") for hc in range(HC)]
        with tc.tile_pool(name="embp", bufs=3) as ep, \
             tc.tile_pool(name="embps", bufs=4, space="PSUM") as epp:
            for chunk in range(16):
                xg = ep.tile([128, H], F32, name="xg", tag="xg")
                nc.gpsimd.indirect_dma_start(
                    out=xg[:, :], out_offset=None, in_=embed_e[:, :],
                    in_offset=bass.IndirectOffsetOnAxis(
                        ap=tok_pm_s[:, chunk:chunk + 1], axis=0))
                for hc in range(HC):
                    pt = epp.tile([128, 128], F32, name="pt", tag="pt")
                    nc.tensor.transpose(out=pt, in_=xg[:, hc * 128:(hc + 1) * 128],
                                        identity=ident_f)
                    nc.vector.tensor_copy(
                        out=x_fm[hc][:, chunk * 128:(chunk + 1) * 128], in_=pt)

        # ---------------- helpers ----------------
        def compute_inv(pool, tag):
            """1/rms of x_fm columns -> bf16 [128, T] broadcast tile (in pool)."""
            with tc.tile_pool(name=f"ssq{tag}", bufs=2) as sp, \
                 tc.tile_pool(name=f"ssqp{tag}", bufs=1, space="PSUM") as spp:
                ssp = spp.tile([1, T], F32, name="ssp", tag="ssp")
                for hc in range(HC):
                    for qc in range(NT):
                        sl = slice(qc * 512, (qc + 1) * 512)
                        sq = sp.tile([128, 512], BF16, name="sq", tag="sq")
                        nc.vector.tensor_mul(out=sq, in0=x_fm[hc][:, sl], in1=x_fm[hc][:, sl])
                        nc.tensor.matmul(out=ssp[0:1, sl], lhsT=ones_b, rhs=sq,
                                         start=(hc == 0), stop=(hc == HC - 1))
                rms = sp.tile([1, T], BF16, name="rms", tag="rms", bufs=1)
                nc.scalar.activation(rms, ssp[0:1, :], Sqrt, bias=eps_t[0:1, :], scale=1.0 / H)
                inv = sp.tile([1, T], BF16, name="inv", tag="inv", bufs=1)
                nc.vector.reciprocal(inv, rms)
                invbc = pool.tile([128, T], BF16, name="invbc", tag=f"invbc{tag}", bufs=1)
                nc.gpsimd.partition_broadcast(invbc, inv)
            return invbc

        def cast_w(pool, dram_ap, g_col=None, tag="wc", stage="wst", bufs=1):
            n = dram_ap.shape[-1]
            wf = pool.tile([128, n], F32, name="wf", tag=stage, bufs=3)
            nc.sync.dma_start(out=wf, in_=dram_ap)
            wb = pool.tile([128, n], BF16, name="wb", tag=tag, bufs=bufs)
            if g_col is not None:
                nc.vector.tensor_scalar(wb, wf, g_col, None, Alu.mult)
            else:
                nc.vector.tensor_copy(out=wb, in_=wf)
            return wb

        def g_cols(pool, ln_row, tag):
            cols = []
            for hc in range(HC):
                gc = pool.tile([128, 1], F32, name=f"g{tag}{hc}", tag=f"g{tag}{hc}")
                nc.sync.dma_start(
                    out=gc, in_=ln_row[hc * 128:(hc + 1) * 128].rearrange("(p o) -> p o", o=1))
                cols.append(gc)
            return cols

        # ---------------- transformer layers ----------------
        for l in range(NL):
            with tc.tile_pool(name=f"lay{l}", bufs=1) as wp:
                g1 = g_cols(wp, ln1_e[l], "g1")
                q_r = wp.tile([128, T], BF16, name="q_r", tag="q_r")
                k_r = wp.tile([128, T], BF16, name="k_r", tag="k_r")
                v_tok = wp.tile([128, HPC, B, 8, 64], BF16, name="v_tok", tag="v_tok")
                attn = wp.tile([128, T], BF16, name="attn", tag="attn")

                # ---- rmsnorm1 + QKV + rope + v transposes, chunked over T ----
                with tc.tile_pool(name="np1", bufs=2) as np1:
                    invbc = compute_inv(np1, "1")
                    wbs = {}
                    with tc.tile_pool(name="pp1", bufs=1, space="PSUM") as pp1, \
                         tc.tile_pool(name="vtp", bufs=2, space="PSUM") as vtp:
                        for tcn in range(NT):
                            sl = slice(tcn * 512, (tcn + 1) * 512)
                            psq = pp1.tile([128, 512], F32, name="psq", tag="psq", bufs=2)
                            psk = pp1.tile([128, 512], F32, name="psk", tag="psk", bufs=2)
                            psv = pp1.tile([128, 512], F32, name="psv", tag="psv", bufs=2)
                            for hc in range(HC):
                                nch = np1.tile([128, 512], BF16, name="nch", tag="nch")
                                nc.vector.scalar_tensor_tensor(
                                    out=nch, in0=x_fm[hc][:, sl], scalar=1.0,
                                    in1=invbc[:, sl], op0=Alu.mult, op1=Alu.mult)
                                if tcn == 0:
                                    wbs[hc] = (
                                        cast_w(wp, wq_e[l, hc * 128:(hc + 1) * 128, :],
                                               g1[hc], tag=f"wq{hc}"),
                                        cast_w(wp, wk_e[l, hc * 128:(hc + 1) * 128, :],
                                               g1[hc], tag=f"wk{hc}"),
                                        cast_w(wp, wv_e[l, hc * 128:(hc + 1) * 128, :],
                                               g1[hc], tag=f"wv{hc}"))
                                wq_b, wk_b, wv_b = wbs[hc]
                                st = (hc == 0); sp_ = (hc == HC - 1)
                                nc.tensor.matmul(out=psq, lhsT=wq_b, rhs=nch, start=st, stop=sp_)
                                nc.tensor.matmul(out=psk, lhsT=wk_b, rhs=nch, start=st, stop=sp_)
                                nc.tensor.matmul(out=psv, lhsT=wv_b, rhs=nch, start=st, stop=sp_)
                            qc_ = np1.tile([128, 512], BF16, name="qc_", tag="qc_")
                            kc_ = np1.tile([128, 512], BF16, name="kc_", tag="kc_")
                            vc_ = np1.tile([128, 512], BF16, name="vc_", tag="vc_")
                            nc.vector.tensor_copy(out=qc_, in_=psq)
                            nc.vector.tensor_copy(out=kc_, in_=psk)
                            nc.vector.tensor_copy(out=vc_, in_=psv)
                            # rope on this chunk
                            for src, dst in ((qc_, q_r), (kc_, k_r)):
                                sh = np1.tile([128, 512], BF16, name="sh", tag="sh")
                                for h in range(HPC):
                                    o = h * 64
                                    nc.sync.dma_start(out=sh[o:o + 32, :], in_=src[o + 32:o + 64, :])
                                    nc.sync.dma_start(out=sh[o + 32:o + 64, :], in_=src[o:o + 32, :])
                                t1_ = np1.tile([128, 512], BF16, name="t1r", tag="t1r")
                                nc.vector.tensor_mul(out=t1_, in0=src, in1=cos_s[:, sl])
                                t2_ = np1.tile([128, 512], BF16, name="t2r", tag="t2r")
                                nc.vector.tensor_mul(out=t2_, in0=sh, in1=sin_s[:, sl])
                                nc.vector.tensor_add(out=dst[:, sl], in0=t1_, in1=t2_)
                            # v transposes for this chunk (4 kc's x 2 heads)
                            b = tcn // 2
                            for h in range(HPC):
                                for j in range(4):
                                    kcx = (tcn % 2) * 4 + j
                                    pt = vtp.tile([128, 64], BF16, name="ptv", tag="ptv")
                                    nc.tensor.transpose(
                                        out=pt, in_=vc_[h * 64:(h + 1) * 64, j * 128:(j + 1) * 128],
                                        identity=ident_b[h * 64:(h + 1) * 64, h * 64:(h + 1) * 64])
                                    nc.vector.tensor_copy(out=v_tok[:, h, b, kcx, :], in_=pt)

                # ---- attention per b; both heads issued adjacently so the PE
                #      packs them: scores K=64 row groups (0,*)/(64,*), PV M=64
                #      col groups via a shared [128,512] PSUM out ----
                with tc.tile_pool(name="atp", bufs=2) as ap_, \
                     tc.tile_pool(name="atpp", bufs=1, space="PSUM") as app:
                    for b in range(B):
                        dps = [app.tile([1, L], F32, name=f"dps{h}", tag=f"dps{h}")
                               for h in range(HPC)]
                        po = [app.tile([128, 512], F32, name=f"po{qc}", tag=f"po{qc}")
                              for qc in range(2)]
                        for kc in range(8):
                            bia = ap_.tile([128, L], BF16, name="bia", tag="bia", bufs=3)
                            nc.sync.dma_start(out=bia, in_=bias_d[b, kc])
                            p_h = [ap_.tile([128, L], BF16, name=f"p_h{h}", tag=f"p_h{h}",
                                            bufs=2) for h in range(HPC)]
                            ksl = slice(b * L + kc * 128, b * L + (kc + 1) * 128)
                            for qc in range(2):
                                qsl = slice(b * L + qc * 512, b * L + (qc + 1) * 512)
                                ps = [app.tile([128, 512], F32, name=f"ps_s{h}",
                                               tag=f"ps_s{h}") for h in range(HPC)]
                                for h in range(HPC):
                                    hsl = slice(h * 64, (h + 1) * 64)
                                    nc.tensor.matmul(out=ps[h], lhsT=k_r[hsl, ksl],
                                                     rhs=q_r[hsl, qsl],
                                                     start=True, stop=True)
                                for h in range(HPC):
                                    sb = ap_.tile([128, 512], F32, name="sb_s",
                                                  tag="sb_s", bufs=3)
                                    nc.vector.scalar_tensor_tensor(
                                        out=sb, in0=ps[h], scalar=0.125,
                                        in1=bia[:, qc * 512:(qc + 1) * 512],
                                        op0=Alu.mult, op1=Alu.add)
                                    nc.scalar.activation(
                                        p_h[h][:, qc * 512:(qc + 1) * 512], sb, Exp)
                            for qc in range(2):
                                qs2 = slice(qc * 512, (qc + 1) * 512)
                                for h in range(HPC):
                                    nc.tensor.matmul(
                                        out=po[qc][h * 64:(h + 1) * 64, :],
                                        lhsT=v_tok[:, h, b, kc, :], rhs=p_h[h][:, qs2],
                                        start=(kc == 0), stop=(kc == 7))
                                for h in range(HPC):
                                    nc.tensor.matmul(
                                        out=dps[h][0:1, qs2], lhsT=ones_b,
                                        rhs=p_h[h][:, qs2],
                                        start=(kc == 0), stop=(kc == 7))
                        dinv_bc = ap_.tile([128, L], BF16, name="dinv_bc", tag="dinv_bc")
                        for h in range(HPC):
                            dinv = ap_.tile([1, L], BF16, name="dinv", tag="dinv")
                            nc.vector.reciprocal(dinv, dps[h][0:1, :])
                            nc.gpsimd.partition_broadcast(
                                dinv_bc[h * 64:(h + 1) * 64, :], dinv)
                        for qc in range(2):
                            nc.vector.scalar_tensor_tensor(
                                out=attn[:, b * L + qc * 512:b * L + (qc + 1) * 512],
                                in0=po[qc], scalar=1.0,
                                in1=dinv_bc[:, qc * 512:(qc + 1) * 512],
                                op0=Alu.mult, op1=Alu.mult)

                # ---- Wo partial -> AllReduce -> residual ----
                ar_in = dram.tile([HC, 128, T], BF16, name=f"ar_in_a{l}", tag="ar_in_a", bufs=2)
                with tc.tile_pool(name="wop", bufs=2) as wop, \
                     tc.tile_pool(name="wopp", bufs=3, space="PSUM") as wpp:
                    for oc in range(HC):
                        wb = cast_w(wop, wo_e[l, :, oc * 128:(oc + 1) * 128], None,
                                    tag="wo", bufs=2)
                        for tcn in range(NT):
                            sl = slice(tcn * 512, (tcn + 1) * 512)
                            ps = wpp.tile([128, 512], F32, name="ps_wo", tag="ps_wo")
                            nc.tensor.matmul(out=ps, lhsT=wb, rhs=attn[:, sl],
                                             start=True, stop=True)
                            ob = wop.tile([128, 512], BF16, name="ob", tag="ob", bufs=3)
                            nc.vector.tensor_copy(out=ob, in_=ps)
                            nc.sync.dma_start(out=ar_in[oc, :, sl], in_=ob)
                ar_out = dram.tile([HC, 128, T], BF16, name=f"ar_out_a{l}", tag="ar_out_a",
                                   addr_space="Shared", bufs=2)
                nc.gpsimd.collective_compute(
                    "AllReduce", Alu.add, replica_groups=RG,
                    ins=[ar_in[:].opt()], outs=[ar_out[:].opt()])
                with tc.tile_pool(name="resp", bufs=3) as rp:
                    for hc in range(HC):
                        rsb = rp.tile([128, T], BF16, name="rsb", tag="rsb")
                        nc.sync.dma_start(out=rsb, in_=ar_out[hc])
                        nc.vector.tensor_add(out=x_fm[hc], in0=x_fm[hc], in1=rsb)

                # ---- rmsnorm2 + MLP ----
                g2 = g_cols(wp, ln2_e[l], "g2")
                ar_in2 = dram.tile([HC, 128, T], BF16, name=f"ar_in_m{l}", tag="ar_in_m", bufs=2)
                with tc.tile_pool(name="np2", bufs=2) as np2:
                    invbc2 = compute_inv(np2, "2")
                    wgbs, wubs, wdbs = {}, {}, {}
                    with tc.tile_pool(name="pp2", bufs=1, space="PSUM") as pp2:
                        for tcn in range(NT):
                            sl = slice(tcn * 512, (tcn + 1) * 512)
                            hm = np2.tile([128, FS // 128, 512], BF16, name="hm", tag="hm")
                            n2 = [None] * HC
                            for hc in range(HC):
                                n2[hc] = np2.tile([128, 512], BF16, name="n2",
                                                  tag=f"n2_{hc}", bufs=1)
                                nc.vector.scalar_tensor_tensor(
                                    out=n2[hc], in0=x_fm[hc][:, sl], scalar=1.0,
                                    in1=invbc2[:, sl], op0=Alu.mult, op1=Alu.mult)
                            for fc in range(FS // 128):
                                psg = pp2.tile([128, 512], F32, name="psg", tag="psg", bufs=2)
                                psu = pp2.tile([128, 512], F32, name="psu", tag="psu", bufs=2)
                                for hc in range(HC):
                                    if tcn == 0:
                                        wgbs[(fc, hc)] = cast_w(
                                            wp, wg_e[l, hc * 128:(hc + 1) * 128,
                                                     fc * 128:(fc + 1) * 128],
                                            g2[hc], tag=f"wg{fc}_{hc}")
                                        wubs[(fc, hc)] = cast_w(
                                            wp, wu_e[l, hc * 128:(hc + 1) * 128,
                                                     fc * 128:(fc + 1) * 128],
                                            g2[hc], tag=f"wu{fc}_{hc}")
                                    st = (hc == 0); sp_ = (hc == HC - 1)
                                    nc.tensor.matmul(out=psg, lhsT=wgbs[(fc, hc)], rhs=n2[hc],
                                                     start=st, stop=sp_)
                                    nc.tensor.matmul(out=psu, lhsT=wubs[(fc, hc)], rhs=n2[hc],
                                                     start=st, stop=sp_)
                                sg = np2.tile([128, 512], BF16, name="sg", tag="sg")
                                nc.scalar.activation(sg, psg, Sigmoid)
                                tg = np2.tile([128, 512], BF16, name="tg", tag="tg")
                                nc.vector.tensor_tensor(out=tg, in0=sg, in1=psg, op=Alu.mult)
                                nc.vector.tensor_tensor(out=hm[:, fc, :], in0=tg, in1=psu,
                                                        op=Alu.mult)
                            for oc in range(HC):
                                ps = pp2.tile([128, 512], F32, name="ps_wd", tag="ps_wd", bufs=2)
                                for fc in range(FS // 128):
                                    if tcn == 0:
                                        wdbs[(oc, fc)] = cast_w(
                                            wp, wd_e[l, fc * 128:(fc + 1) * 128,
                                                     oc * 128:(oc + 1) * 128],
                                            None, tag=f"wd{oc}_{fc}")
                                    nc.tensor.matmul(out=ps, lhsT=wdbs[(oc, fc)],
                                                     rhs=hm[:, fc, :],
                                                     start=(fc == 0), stop=(fc == FS // 128 - 1))
                                ob = np2.tile([128, 512], BF16, name="ob2", tag="ob2", bufs=3)
                                nc.vector.tensor_copy(out=ob, in_=ps)
                                nc.sync.dma_start(out=ar_in2[oc, :, sl], in_=ob)
                ar_out2 = dram.tile([HC, 128, T], BF16, name=f"ar_out_m{l}", tag="ar_out_m",
                                    addr_space="Shared", bufs=2)
                nc.gpsimd.collective_compute(
                    "AllReduce", Alu.add, replica_groups=RG,
                    ins=[ar_in2[:].opt()], outs=[ar_out2[:].opt()])
                with tc.tile_pool(name="resp2", bufs=3) as rp:
                    for hc in range(HC):
                        rsb = rp.tile([128, T], BF16, name="rsb2", tag="rsb")
                        nc.sync.dma_start(out=rsb, in_=ar_out2[hc])
                        nc.vector.tensor_add(out=x_fm[hc], in0=x_fm[hc], in1=rsb)

        # ---------------- final norm into head-scope tiles ----------------
        heads = ctx.enter_context(tc.tile_pool(name="heads", bufs=1, side="right"))
        n_fin = [heads.tile([128, T], BF16, name=f"nf{hc}") for hc in range(HC)]
        with tc.tile_pool(name="fnp", bufs=2) as fp_:
            invbcf = compute_inv(fp_, "f")
            for hc in range(HC):
                nc.vector.scalar_tensor_tensor(
                    out=n_fin[hc], in0=x_fm[hc], scalar=1.0, in1=invbcf,
                    op0=Alu.mult, op1=Alu.mult)
        trunk_ctx.close()

        gf = g_cols(heads, fln_e, "gf")
        te_sb = [heads.tile([128, TS], BF16, name=f"te{c4}") for c4 in range(TD // 128)]
        with tc.tile_pool(name="tep", bufs=2) as tep:
            ones512 = tep.tile([128, S], BF16, name="ones512", tag="ones512", bufs=1)
            nc.vector.memset(ones512, 1.0)
            for c4 in range(TD // 128):
                for b in range(B):
                    tecol = tep.tile([128, 1], F32, name="tecol", tag="tecol")
                    nc.sync.dma_start(
                        out=tecol,
                        in_=te_e[b, c4 * 128:(c4 + 1) * 128].rearrange("(p o) -> p o", o=1))
                    nc.vector.tensor_scalar(te_sb[c4][:, b * S:(b + 1) * S],
                                            ones512, tecol, None, Alu.mult)

        def xc_rhs(cc, tc2):
            if cc < HC:
                return n_fin[cc][:, (2 * tc2 + 1) * 512:(2 * tc2 + 2) * 512]
            return te_sb[cc - HC][:, tc2 * 512:(tc2 + 1) * 512]

        def head_h1(pool, ppool, w1_e, b1_e, nout, tag):
            outs = []
            b1s = pool.tile([128, nout // 128], F32, name=f"b1s{tag}", tag=f"b1s{tag}", bufs=1)
            nc.sync.dma_start(out=b1s, in_=b1_e[:].rearrange("(oc p) -> p oc", p=128))
            for oc in range(nout // 128):
                dst = heads.tile([128, TS], BF16, name=f"h1{tag}{oc}")
                for tc2 in range(2):
                    ps = ppool.tile([128, 512], F32, name="ps_h1", tag="ps_h1", bufs=2)
                    for cc in range(12):
                        w_ap = w1_e[cc * 128:(cc + 1) * 128, oc * 128:(oc + 1) * 128]
                        wb = cast_w(pool, w_ap, gf[cc] if cc < HC else None,
                                    tag=f"w1_{cc}", bufs=2)
                        nc.tensor.matmul(out=ps, lhsT=wb, rhs=xc_rhs(cc, tc2),
                                         start=(cc == 0), stop=(cc == 11))
                    s1 = pool.tile([128, 512], BF16, name="s1h", tag="s1h", bufs=2)
                    nc.scalar.activation(s1, ps, Sigmoid, bias=b1s[:, oc:oc + 1])
                    pre = pool.tile([128, 512], F32, name="preh", tag="preh", bufs=2)
                    nc.vector.tensor_scalar(pre, ps, b1s[:, oc:oc + 1], None, Alu.add)
                    nc.vector.tensor_tensor(out=dst[:, tc2 * 512:(tc2 + 1) * 512],
                                            in0=pre, in1=s1, op=Alu.mult)
                outs.append(dst)
            return outs

        with tc.tile_pool(name="h1p", bufs=2) as h1p, \
             tc.tile_pool(name="h1pp", bufs=1, space="PSUM") as h1pp:
            h1i = head_h1(h1p, h1pp, insw1_e, insb1_e, 128, "i")[0]
            h1s = head_h1(h1p, h1pp, subw1_e, subb1_e, 128, "s")[0]
            h2r = head_h1(h1p, h1pp, ratew1_e, rateb1_e, TD, "r")

        def gather_h2(h1_tile, tag):
            ag_in = dram.tile([128, TS], BF16, name=f"ag_in{tag}", tag="ag_in", bufs=2)
            nc.sync.dma_start(out=ag_in, in_=h1_tile)
            ag_out = dram.tile([NC_, 128, TS], BF16, name=f"ag_out{tag}", tag="ag_out",
                               addr_space="Shared", bufs=2)
            nc.gpsimd.collective_compute(
                "AllGather", Alu.bypass, replica_groups=RG,
                ins=[ag_in[:].opt()], outs=[ag_out[:].opt()])
            tiles = []
            for hc in range(HC):
                tl = heads.tile([128, TS], BF16, name=f"h2{tag}{hc}")
                nc.sync.dma_start(out=tl, in_=ag_out[hc])
                tiles.append(tl)
            return tiles

        h2i = gather_h2(h1i, "i")
        h2s = gather_h2(h1s, "s")

        # ---- rate head ----
        with tc.tile_pool(name="rrp", bufs=2) as rrp, \
             tc.tile_pool(name="rrpp", bufs=2, space="PSUM") as rrpp:
            rw2 = rrp.tile([128, TD // 128, 3], BF16, name="rw2", tag="rw2", bufs=1)
            rwf = rrp.tile([128, TD // 128, 3], F32, name="rwf", tag="rwf", bufs=1)
            nc.sync.dma_start(out=rwf, in_=ratew2_e[:, :].rearrange("(fc p) n -> p fc n", p=128))
            nc.vector.tensor_copy(out=rw2, in_=rwf)
            rb2 = rrp.tile([1, 3], BF16, name="rb2", tag="rb2", bufs=1)
            rb2f = rrp.tile([1, 3], F32, name="rb2f", tag="rb2f", bufs=1)
            nc.sync.dma_start(out=rb2f, in_=rateb2_e[:, :])
            nc.vector.tensor_copy(out=rb2, in_=rb2f)
            for tt in range(8):
                ps = rrpp.tile([128, 3], F32, name="ps_r", tag="ps_r")
                for fc in range(TD // 128):
                    nc.tensor.matmul(out=ps, lhsT=h2r[fc][:, tt * 128:(tt + 1) * 128],
                                     rhs=rw2[:, fc, :], start=(fc == 0), stop=False)
                nc.tensor.matmul(out=ps, lhsT=onesrow_b, rhs=rb2, start=False, stop=True)
                ex_ = rrp.tile([128, 3], F32, name="ex_", tag="ex_")
                nc.scalar.activation(ex_, ps, Exp)
                e1_ = rrp.tile([128, 3], F32, name="e1_", tag="e1_")
                nc.vector.tensor_scalar(e1_, ex_, 1.0, None, Alu.add)
                ro = rrp.tile([128, 3], F32, name="ro", tag="ro")
                nc.scalar.activation(ro, e1_, mybir.ActivationFunctionType.Ln)
                rom = rrp.tile([128, 3], F32, name="rom", tag="rom")
                nc.vector.tensor_scalar(rom, ro, m_pm_s[:, tt:tt + 1], None, Alu.mult)
                nc.sync.dma_start(out=rates_o[tt * 128:(tt + 1) * 128, :], in_=rom)

        # ---- ins/sub logits: exp chunks -> DRAM, sums -> one tiny AllReduce --
        NVC = 8
        VC = VS // NVC  # 500
        sums = consts.tile([128, 16], F32, name="sums")
        est_d = dram.tile([2, 8, 128, VS], BF16, name="est_d", tag="est_d")
        for hd_i, (h2t, w2_e, b2_e, tagh) in enumerate(
                ((h2i, insw2_e, insb2_e, "i"), (h2s, subw2_e, subb2_e, "s"))):
            with tc.tile_pool(name=f"lg{tagh}", bufs=2) as lp_, \
                 tc.tile_pool(name=f"lgp{tagh}", bufs=3, space="PSUM") as lpp:
                b2s = lp_.tile([1, VS], BF16, name="b2s", tag="b2s", bufs=1)
                for vc in range(NVC):
                    b2f = lp_.tile([1, VC], F32, name="b2f", tag="b2f", bufs=2)
                    nc.sync.dma_start(out=b2f, in_=b2_e[:, vc * VC:(vc + 1) * VC])
                    nc.vector.tensor_copy(out=b2s[:, vc * VC:(vc + 1) * VC], in_=b2f)
                part = lp_.tile([128, 8, NVC], F32, name="part", tag="part", bufs=1)
                for vc in range(NVC):
                    w2b = [cast_w(lp_, w2_e[h2c * 128:(h2c + 1) * 128, vc * VC:(vc + 1) * VC],
                                  None, tag=f"w2_{h2c}", bufs=2) for h2c in range(HC)]
                    for tt in range(8):
                        ps = lpp.tile([128, VC], F32, name="ps_l", tag="ps_l")
                        for h2c in range(HC):
                            nc.tensor.matmul(out=ps, lhsT=h2t[h2c][:, tt * 128:(tt + 1) * 128],
                                             rhs=w2b[h2c], start=(h2c == 0), stop=False)
                        nc.tensor.matmul(out=ps, lhsT=onesrow_b,
                                         rhs=b2s[:, vc * VC:(vc + 1) * VC],
                                         start=False, stop=True)
                        ech = lp_.tile([128, VC], BF16, name="ech", tag="ech", bufs=3)
                        nc.scalar.activation(ech, ps, Exp,
                                             accum_out=part[:, tt, vc:vc + 1])
                        nc.sync.dma_start(out=est_d[hd_i, tt, :, vc * VC:(vc + 1) * VC],
                                          in_=ech)
                for tt in range(8):
                    nc.vector.reduce_sum(out=sums[:, hd_i * 8 + tt:hd_i * 8 + tt + 1],
                                         in_=part[:, tt, :], axis=AX)

        sar_in = dram.tile([128, 16], F32, name="sar_in", tag="sar_in")
        nc.sync.dma_start(out=sar_in, in_=sums)
        sar_out = dram.tile([128, 16], F32, name="sar_out", tag="sar_out",
                            addr_space="Shared")
        nc.gpsimd.collective_compute(
            "AllReduce", Alu.add, replica_groups=RG,
            ins=[sar_in[:].opt()], outs=[sar_out[:].opt()])
        with tc.tile_pool(name="finp", bufs=3) as fnp:
            sums_g = fnp.tile([128, 16], F32, name="sums_g", tag="sums_g", bufs=1)
            nc.sync.dma_start(out=sums_g, in_=sar_out)
            dinv_all = fnp.tile([128, 16], F32, name="dinv_all", tag="dinv_all", bufs=1)
            nc.vector.reciprocal(dinv_all, sums_g)
            dm = fnp.tile([128, 16], F32, name="dm", tag="dm", bufs=1)
            for hd_i in range(2):
                nc.vector.tensor_tensor(out=dm[:, hd_i * 8:(hd_i + 1) * 8],
                                        in0=dinv_all[:, hd_i * 8:(hd_i + 1) * 8],
                                        in1=m_pm_s, op=Alu.mult)
            for hd_i, out_e in ((0, ins_o), (1, sub_o)):
                for tt in range(8):
                    for vc in range(NVC):
                        eb = fnp.tile([128, VC], BF16, name="eb", tag="eb")
                        nc.sync.dma_start(out=eb, in_=est_d[hd_i, tt, :, vc * VC:(vc + 1) * VC])
                        fo = fnp.tile([128, VC], F32, name="fo", tag="fo")
                        nc.vector.tensor_scalar(fo, eb,
                                                dm[:, hd_i * 8 + tt:hd_i * 8 + tt + 1],
                                                None, Alu.mult)
                        nc.sync.dma_start(
                            out=out_e[tt * 128:(tt + 1) * 128, vc * VC:(vc + 1) * VC], in_=fo)
        ctx.close()
    nc.finalize()
    return nc


# -------------------------------------------------------------------- driver
_CACHE = {}


def kernel(**inputs):
    import os
    from concourse.bass_utils import run_bass_kernel_spmd
    pad_tok, per_core = host_prep(inputs)
    if pad_tok not in _CACHE:
        _CACHE[pad_tok] = build(pad_tok)
    nc = _CACHE[pad_tok]
    trace = bool(int(os.environ.get("BASS_KERNEL_TRACE", "0")))
    res = run_bass_kernel_spmd(nc, per_core, core_ids=list(range(NC_)), trace=trace)
    global LAST_RESULT
    LAST_RESULT = res
    outs = res.results
    rates = np.asarray(outs[0]["rates_out"]).reshape(B, S, 3).astype(np.float32)
    ins = np.concatenate([np.asarray(outs[c]["ins_out"]) for c in range(NC_)],
                         axis=1).reshape(B, S, V).astype(np.float32)
    sub = np.concatenate([np.asarray(outs[c]["sub_out"]) for c in range(NC_)],
                         axis=1).reshape(B, S, V).astype(np.float32)
    return rates, ins, sub
